# revision 5
# baseline (speedup 1.0000x reference)
"""MoE layer (top-2 of 8 experts, SwiGLU) on 8 Trainium2 NeuronCores.

Strategy: expert parallelism. The router (softmax top-2 over a [8192, 8]
logit matrix) runs on the host in numpy; tokens are gathered per expert,
padded to a common capacity C, and each core runs one expert's three GEMMs
with its weights resident in SBUF (bf16 operands, fp32 PSUM accumulation).
The host applies the gate weights during the scatter-add back to the full
output, so padded rows simply contribute nothing.

Device layout is feature-on-partition / token-on-free:
  G.T = Wg.T.T @ X.T   per (H-chunk, D-chunk) tile, accumulated over D
  U.T = W1.T.T @ X.T
  h   = silu(G) * U    (ScalarE silu, VectorE multiply, bf16 result)
  Y.T = W2.T.T @ h     accumulated over H-chunks
"""

import numpy as np
import ml_dtypes

B, S, D = 2, 4096, 1024
E, H, TOPK = 8, 2736, 2
T = B * S
HP = 2816  # H padded to a multiple of 128
KD = D // 128  # 8 contraction chunks over D
KH = HP // 128  # 22 contraction chunks over padded H
N_CORES = 8
CT = 512  # token tile (free dim per matmul, one PSUM bank of fp32)

_BF16 = ml_dtypes.bfloat16


def _install_drain_patch():
    """walrus in this image rejects any instruction carrying >1 sync wait
    ("Too many sync wait commands"). Split waits: every instruction keeps one
    wait; extra waits ride dedicated NoOps inserted just before it on the
    same engine. Applies to the Tile-lowered stream and to the tail drain."""
    import concourse.mybir as mybir
    import concourse.tile as tile
    from concourse.vector_clock import ScopedClock

    if getattr(tile.TileContext, "_drain_patch_installed", False):
        return

    _orig_lower = tile.TileContext._lower_ordered_insts

    def _split_lower(self, ordered):
        nc = self.nc
        for bb_name, insts in ordered.items():
            new = []
            for inst in insts:
                si = inst.sync_info
                ow = list(si.on_wait) if si is not None and si.on_wait else []
                if len(ow) > 1:
                    scopes = self._inst_to_scopes.get(inst.name, ())
                    for w in ow[:-1]:
                        nop = mybir.InstNoOp(
                            name=nc.get_next_instruction_name(),
                            engine=inst.engine,
                            ins=[],
                            outs=[],
                            sync_info=mybir.SyncInfo(on_wait=[w], on_update=[]),
                            bass_nofuse=True,
                        )
                        if scopes:
                            self._inst_to_scopes[nop.name] = scopes
                        new.append(nop)
                    ou = list(si.on_update) if si.on_update else []
                    inst.sync_info = mybir.SyncInfo(on_wait=[ow[-1]], on_update=ou)
                new.append(inst)
            ordered[bb_name] = new
        return _orig_lower(self, ordered)

    tile.TileContext._lower_ordered_insts = _split_lower

    def _patched(self, tick_clock, wait_clock):
        nc = self.nc
        nops = [nc.sync.nop(nofuse=True) for _ in range(30)]
        drain_inst = nc.sync.drain()
        wait_clock.add_sem_waits(
            drain_inst.ins, ScopedClock({None: tick_clock.global_clock})
        )
        si = drain_inst.ins.sync_info
        ow = list(si.on_wait) if si is not None and si.on_wait else []
        if len(ow) > 1:
            assert len(ow) <= 1 + len(nops), f"drain needs {len(ow)} waits"
            for i, w in enumerate(ow[:-1]):
                nops[i].ins.sync_info = mybir.SyncInfo(on_wait=[w], on_update=[])
            ou = list(si.on_update) if si.on_update else []
            drain_inst.ins.sync_info = mybir.SyncInfo(on_wait=[ow[-1]], on_update=ou)
        nc.all_engine_barrier()
        assert self.sems is not None
        popped = nc._tile_sem_poison_stack.pop()
        assert popped is self._sem_poison
        nc.clear_and_free_semaphores(list(self.sems.allocated().values()))
        nc.all_engine_barrier()

    tile.TileContext._drain_and_barrier = _patched
    tile.TileContext._drain_patch_installed = True


def _token_tiles(C):
    """Near-equal tile sizes (multiples of 64, each <=512) so the matmul
    moving dim stays large enough to hide LDWEIGHTS."""
    n = -(-C // CT)
    base = (C // n) // 64 * 64
    sizes = [base] * n
    extra = (C - base * n) // 64
    for i in range(extra):
        sizes[i] += 64
    tiles = []
    t0 = 0
    for ct in sizes:
        tiles.append((t0, ct))
        t0 += ct
    assert t0 == C, (C, sizes)
    return tiles


_PROGRAM_CACHE = {}


def _build_program(C):
    """One SPMD program: expert FFN over C (padded) tokens."""
    if C in _PROGRAM_CACHE:
        return _PROGRAM_CACHE[C]

    _install_drain_patch()
    import concourse.bass as bass
    import concourse.mybir as mybir
    import concourse.tile as tile

    bf16 = mybir.dt.bfloat16
    f32 = mybir.dt.float32

    nc = bass.Bass()
    xT = nc.declare_dram_parameter("xT", [D, C], bf16, isOutput=False)
    wgT = nc.declare_dram_parameter("wgT", [D, HP], bf16, isOutput=False)
    w1T = nc.declare_dram_parameter("w1T", [D, HP], bf16, isOutput=False)
    w2T = nc.declare_dram_parameter("w2T", [HP, D], bf16, isOutput=False)
    yT = nc.declare_dram_parameter("yT", [D, C], f32, isOutput=True)

    with tile.TileContext(nc) as tc:
        with (
            tc.tile_pool(name="wpool", bufs=1) as wpool,
            tc.tile_pool(name="xpool", bufs=2) as xpool,
            tc.tile_pool(name="hpool", bufs=1) as hpool,
            tc.tile_pool(name="gpool", bufs=3) as gpool,
            tc.tile_pool(name="ypool", bufs=3) as ypool,
            tc.tile_pool(name="pg", bufs=2, space="PSUM") as pg,
            tc.tile_pool(name="pu", bufs=2, space="PSUM") as pu,
            tc.tile_pool(name="py", bufs=2, space="PSUM") as py,
        ):
            # Resident weights. wg/w1 are loaded in hk-major column-slice
            # groups (4 H-chunks = 512 cols per DMA) so the first token
            # tile's GEMMs can start as soon as the first slices land
            # instead of stalling ~40us on the full 11.5MB wg+w1 load.
            HKG = 4  # H-chunks per slice group
            n_grp = -(-KH // HKG)
            wg_s = [[None] * n_grp for _ in range(KD)]
            w1_s = [[None] * n_grp for _ in range(KD)]
            for g in range(n_grp):
                c0 = g * HKG * 128
                cw = min(HKG * 128, HP - c0)
                for d in range(KD):
                    t = wpool.tile([128, cw], bf16, tag=f"wg{d}_{g}")
                    nc.sync.dma_start(
                        t[:], wgT[d * 128:(d + 1) * 128, c0:c0 + cw]
                    )
                    wg_s[d][g] = t
                    t = wpool.tile([128, cw], bf16, tag=f"w1{d}_{g}")
                    nc.sync.dma_start(
                        t[:], w1T[d * 128:(d + 1) * 128, c0:c0 + cw]
                    )
                    w1_s[d][g] = t

            def wg_slice(d, hk):
                return wg_s[d][hk // HKG][:, (hk % HKG) * 128:(hk % HKG + 1) * 128]

            def w1_slice(d, hk):
                return w1_s[d][hk // HKG][:, (hk % HKG) * 128:(hk % HKG + 1) * 128]

            w2_s = []
            for hk in range(KH):
                t = wpool.tile([128, D], bf16, tag=f"w2{hk}")
                nc.sync.dma_start(t[:], w2T[hk * 128:(hk + 1) * 128, :])
                w2_s.append(t)

            for t0, ct in _token_tiles(C):
                x_s = xpool.tile([128, KD * ct], bf16, tag="x")
                for d in range(KD):
                    nc.sync.dma_start(
                        x_s[:, d * ct:(d + 1) * ct],
                        xT[d * 128:(d + 1) * 128, t0:t0 + ct],
                    )

                h_s = hpool.tile([128, KH * ct], bf16, tag="h")
                for hk in range(KH):
                    g_ps = pg.tile([128, ct], f32)
                    u_ps = pu.tile([128, ct], f32)
                    for d in range(KD):
                        nc.tensor.matmul(
                            g_ps[:],
                            wg_slice(d, hk),
                            x_s[:, d * ct:(d + 1) * ct],
                            start=(d == 0),
                            stop=(d == KD - 1),
                        )
                        nc.tensor.matmul(
                            u_ps[:],
                            w1_slice(d, hk),
                            x_s[:, d * ct:(d + 1) * ct],
                            start=(d == 0),
                            stop=(d == KD - 1),
                        )
                    g_tmp = gpool.tile([128, ct], f32, tag="g")
                    nc.scalar.activation(
                        g_tmp[:], g_ps[:], mybir.ActivationFunctionType.Silu
                    )
                    nc.vector.tensor_mul(
                        h_s[:, hk * ct:(hk + 1) * ct], g_tmp[:], u_ps[:]
                    )

                for d in range(KD):
                    y_ps = py.tile([128, ct], f32)
                    for hk in range(KH):
                        nc.tensor.matmul(
                            y_ps[:],
                            w2_s[hk][:, d * 128:(d + 1) * 128],
                            h_s[:, hk * ct:(hk + 1) * ct],
                            start=(hk == 0),
                            stop=(hk == KH - 1),
                        )
                    y_sb = ypool.tile([128, ct], f32, tag="y")
                    nc.vector.tensor_copy(y_sb[:], y_ps[:])
                    nc.sync.dma_start(
                        yT[d * 128:(d + 1) * 128, t0:t0 + ct], y_sb[:]
                    )

    _PROGRAM_CACHE[C] = nc
    return nc


def _route(xf, gate_w):
    """Host router matching the reference: fp32 logits/softmax, top-2."""
    logits = xf @ gate_w.T  # [T, E] fp32
    m = logits.max(axis=1, keepdims=True)
    p = np.exp(logits - m, dtype=np.float32)
    p /= p.sum(axis=1, keepdims=True)
    # softmax is monotonic in logits, so top-2 by probs == top-2 by logits
    top_i = np.argsort(-p, axis=1, kind="stable")[:, :TOPK]  # [T, 2]
    top_p = np.take_along_axis(p, top_i, axis=1)
    gate_weights = top_p / (top_p.sum(axis=1, keepdims=True) + np.float32(1e-8))
    return top_i, gate_weights.astype(np.float32)


def kernel(x, gate_w, Wg, W1, W2):
    from concourse.bass_utils import run_bass_kernel_spmd

    x = np.asarray(x, dtype=np.float32)
    gate_w = np.asarray(gate_w, dtype=np.float32)
    Wg = np.asarray(Wg, dtype=np.float32)
    W1 = np.asarray(W1, dtype=np.float32)
    W2 = np.asarray(W2, dtype=np.float32)

    xf = x.reshape(-1, D)
    top_i, gate_weights = _route(xf, gate_w)

    idx = [None] * E
    wts = [None] * E
    for e in range(E):
        rows, slots = np.nonzero(top_i == e)
        idx[e] = rows
        wts[e] = gate_weights[rows, slots]
    counts = np.array([len(i) for i in idx])
    C = max(128, int(np.ceil(counts.max() / 128)) * 128)

    nc = _build_program(C)

    xf_bf = xf.astype(_BF16)
    in_maps = []
    for e in range(E):
        xT_e = np.zeros((D, C), dtype=_BF16)
        xT_e[:, : counts[e]] = xf_bf[idx[e]].T
        wgT_e = np.zeros((D, HP), dtype=_BF16)
        wgT_e[:, :H] = Wg[e].astype(_BF16).T
        w1T_e = np.zeros((D, HP), dtype=_BF16)
        w1T_e[:, :H] = W1[e].astype(_BF16).T
        w2T_e = np.zeros((HP, D), dtype=_BF16)
        w2T_e[:H, :] = W2[e].astype(_BF16).T
        in_maps.append({"xT": xT_e, "wgT": wgT_e, "w1T": w1T_e, "w2T": w2T_e})

    res = run_bass_kernel_spmd(nc, in_maps, list(range(N_CORES)))

    out = np.zeros((T, D), dtype=np.float32)
    for e in range(E):
        yT_e = res.results[e]["yT"]  # [D, C] fp32
        out[idx[e]] += wts[e][:, None] * yT_e.T[: counts[e]]
    return out.reshape(B, S, D)


# revision 7
# speedup vs baseline: 1.1270x; 1.1270x over previous
"""MoE layer (top-2 of 8 experts, SwiGLU) on 8 Trainium2 NeuronCores.

Strategy: expert parallelism. The router (softmax top-2 over a [8192, 8]
logit matrix) runs on the host in numpy; tokens are gathered per expert,
padded to a common capacity C, and each core runs one expert's three GEMMs
with its weights resident in SBUF (bf16 operands, fp32 PSUM accumulation).
The host applies the gate weights during the scatter-add back to the full
output, so padded rows simply contribute nothing.

Device layout is feature-on-partition / token-on-free:
  G.T = Wg.T.T @ X.T   per (H-chunk, D-chunk) tile, accumulated over D
  U.T = W1.T.T @ X.T
  h   = silu(G) * U    (ScalarE silu, VectorE multiply, bf16 result)
  Y.T = W2.T.T @ h     accumulated over H-chunks
"""

import numpy as np
import ml_dtypes

B, S, D = 2, 4096, 1024
E, H, TOPK = 8, 2736, 2
T = B * S
HP = 2816  # H padded to a multiple of 128
KD = D // 128  # 8 contraction chunks over D
KH = HP // 128  # 22 contraction chunks over padded H
N_CORES = 8
CT = 512  # token tile (free dim per matmul, one PSUM bank of fp32)

_BF16 = ml_dtypes.bfloat16


def _install_drain_patch():
    """walrus in this image rejects any instruction carrying >1 sync wait
    ("Too many sync wait commands"). Split waits: every instruction keeps one
    wait; extra waits ride dedicated NoOps inserted just before it on the
    same engine. Applies to the Tile-lowered stream and to the tail drain."""
    import concourse.mybir as mybir
    import concourse.tile as tile
    from concourse.vector_clock import ScopedClock

    if getattr(tile.TileContext, "_drain_patch_installed", False):
        return

    _orig_lower = tile.TileContext._lower_ordered_insts

    def _split_lower(self, ordered):
        nc = self.nc
        for bb_name, insts in ordered.items():
            new = []
            for inst in insts:
                si = inst.sync_info
                ow = list(si.on_wait) if si is not None and si.on_wait else []
                if len(ow) > 1:
                    scopes = self._inst_to_scopes.get(inst.name, ())
                    for w in ow[:-1]:
                        nop = mybir.InstNoOp(
                            name=nc.get_next_instruction_name(),
                            engine=inst.engine,
                            ins=[],
                            outs=[],
                            sync_info=mybir.SyncInfo(on_wait=[w], on_update=[]),
                            bass_nofuse=True,
                        )
                        if scopes:
                            self._inst_to_scopes[nop.name] = scopes
                        new.append(nop)
                    ou = list(si.on_update) if si.on_update else []
                    inst.sync_info = mybir.SyncInfo(on_wait=[ow[-1]], on_update=ou)
                new.append(inst)
            ordered[bb_name] = new
        return _orig_lower(self, ordered)

    tile.TileContext._lower_ordered_insts = _split_lower

    def _patched(self, tick_clock, wait_clock):
        nc = self.nc
        nops = [nc.sync.nop(nofuse=True) for _ in range(30)]
        drain_inst = nc.sync.drain()
        wait_clock.add_sem_waits(
            drain_inst.ins, ScopedClock({None: tick_clock.global_clock})
        )
        si = drain_inst.ins.sync_info
        ow = list(si.on_wait) if si is not None and si.on_wait else []
        if len(ow) > 1:
            assert len(ow) <= 1 + len(nops), f"drain needs {len(ow)} waits"
            for i, w in enumerate(ow[:-1]):
                nops[i].ins.sync_info = mybir.SyncInfo(on_wait=[w], on_update=[])
            ou = list(si.on_update) if si.on_update else []
            drain_inst.ins.sync_info = mybir.SyncInfo(on_wait=[ow[-1]], on_update=ou)
        nc.all_engine_barrier()
        assert self.sems is not None
        popped = nc._tile_sem_poison_stack.pop()
        assert popped is self._sem_poison
        nc.clear_and_free_semaphores(list(self.sems.allocated().values()))
        nc.all_engine_barrier()

    tile.TileContext._drain_and_barrier = _patched
    tile.TileContext._drain_patch_installed = True


def _token_tiles(C):
    """Near-equal tile sizes (multiples of 64, each <=512) so the matmul
    moving dim stays large enough to hide LDWEIGHTS."""
    n = -(-C // CT)
    base = (C // n) // 64 * 64
    sizes = [base] * n
    extra = (C - base * n) // 64
    for i in range(extra):
        sizes[i] += 64
    tiles = []
    t0 = 0
    for ct in sizes:
        tiles.append((t0, ct))
        t0 += ct
    assert t0 == C, (C, sizes)
    return tiles


_PROGRAM_CACHE = {}


def _build_program(C):
    """One SPMD program: expert FFN over C (padded) tokens."""
    if C in _PROGRAM_CACHE:
        return _PROGRAM_CACHE[C]

    _install_drain_patch()
    import concourse.bass as bass
    import concourse.mybir as mybir
    import concourse.tile as tile

    bf16 = mybir.dt.bfloat16
    f32 = mybir.dt.float32

    nc = bass.Bass()
    xT = nc.declare_dram_parameter("xT", [D, C], bf16, isOutput=False)
    wgT = nc.declare_dram_parameter("wgT", [D, HP], bf16, isOutput=False)
    w1T = nc.declare_dram_parameter("w1T", [D, HP], bf16, isOutput=False)
    w2T = nc.declare_dram_parameter("w2T", [HP, D], bf16, isOutput=False)
    yT = nc.declare_dram_parameter("yT", [D, C], f32, isOutput=True)

    with tile.TileContext(nc) as tc:
        with (
            tc.tile_pool(name="wpool", bufs=1) as wpool,
            tc.tile_pool(name="xpool", bufs=2) as xpool,
            tc.tile_pool(name="hpool", bufs=1) as hpool,
            tc.tile_pool(name="gpool", bufs=3) as gpool,
            tc.tile_pool(name="ypool", bufs=3) as ypool,
            tc.tile_pool(name="pg", bufs=2, space="PSUM") as pg,
            tc.tile_pool(name="pu", bufs=2, space="PSUM") as pu,
            tc.tile_pool(name="py", bufs=2, space="PSUM") as py,
        ):
            tiles = _token_tiles(C)
            x_tiles = {}

            def load_x(t_idx):
                t0, ct = tiles[t_idx]
                x_s = xpool.tile([128, KD * ct], bf16, tag="x")
                for d in range(KD):
                    nc.sync.dma_start(
                        x_s[:, d * ct:(d + 1) * ct],
                        xT[d * 128:(d + 1) * 128, t0:t0 + ct],
                    )
                x_tiles[t_idx] = x_s

            # Token DMAs for tile 0 go first: the first GEMM needs them, and
            # everything issued after the weight block would land ~70us in.
            load_x(0)

            # Resident weights. wg/w1 are loaded in hk-major column-slice
            # groups (4 H-chunks = 512 cols per DMA) so the first token
            # tile's GEMMs can start as soon as the first slices land
            # instead of stalling ~40us on the full 11.5MB wg+w1 load.
            HKG = 4  # H-chunks per slice group
            n_grp = -(-KH // HKG)
            wg_s = [[None] * n_grp for _ in range(KD)]
            w1_s = [[None] * n_grp for _ in range(KD)]
            for g in range(n_grp):
                c0 = g * HKG * 128
                cw = min(HKG * 128, HP - c0)
                for d in range(KD):
                    t = wpool.tile([128, cw], bf16, tag=f"wg{d}_{g}")
                    nc.sync.dma_start(
                        t[:], wgT[d * 128:(d + 1) * 128, c0:c0 + cw]
                    )
                    wg_s[d][g] = t
                    t = wpool.tile([128, cw], bf16, tag=f"w1{d}_{g}")
                    nc.sync.dma_start(
                        t[:], w1T[d * 128:(d + 1) * 128, c0:c0 + cw]
                    )
                    w1_s[d][g] = t

            def wg_slice(d, hk):
                return wg_s[d][hk // HKG][:, (hk % HKG) * 128:(hk % HKG + 1) * 128]

            def w1_slice(d, hk):
                return w1_s[d][hk // HKG][:, (hk % HKG) * 128:(hk % HKG + 1) * 128]

            w2_s = []
            for hk in range(KH):
                t = wpool.tile([128, D], bf16, tag=f"w2{hk}")
                nc.sync.dma_start(t[:], w2T[hk * 128:(hk + 1) * 128, :])
                w2_s.append(t)

            for ti, (t0, ct) in enumerate(tiles):
                if ti + 1 < len(tiles):
                    load_x(ti + 1)
                x_s = x_tiles.pop(ti)

                h_s = hpool.tile([128, KH * ct], bf16, tag="h")
                for hk in range(KH):
                    g_ps = pg.tile([128, ct], f32)
                    u_ps = pu.tile([128, ct], f32)
                    for d in range(KD):
                        nc.tensor.matmul(
                            g_ps[:],
                            wg_slice(d, hk),
                            x_s[:, d * ct:(d + 1) * ct],
                            start=(d == 0),
                            stop=(d == KD - 1),
                        )
                        nc.tensor.matmul(
                            u_ps[:],
                            w1_slice(d, hk),
                            x_s[:, d * ct:(d + 1) * ct],
                            start=(d == 0),
                            stop=(d == KD - 1),
                        )
                    g_tmp = gpool.tile([128, ct], f32, tag="g")
                    nc.scalar.activation(
                        g_tmp[:], g_ps[:], mybir.ActivationFunctionType.Silu
                    )
                    nc.vector.tensor_mul(
                        h_s[:, hk * ct:(hk + 1) * ct], g_tmp[:], u_ps[:]
                    )

                for d in range(KD):
                    y_ps = py.tile([128, ct], f32)
                    for hk in range(KH):
                        nc.tensor.matmul(
                            y_ps[:],
                            w2_s[hk][:, d * 128:(d + 1) * 128],
                            h_s[:, hk * ct:(hk + 1) * ct],
                            start=(hk == 0),
                            stop=(hk == KH - 1),
                        )
                    y_sb = ypool.tile([128, ct], f32, tag="y")
                    nc.vector.tensor_copy(y_sb[:], y_ps[:])
                    nc.sync.dma_start(
                        yT[d * 128:(d + 1) * 128, t0:t0 + ct], y_sb[:]
                    )

    _PROGRAM_CACHE[C] = nc
    return nc


def _route(xf, gate_w):
    """Host router matching the reference: fp32 logits/softmax, top-2."""
    logits = xf @ gate_w.T  # [T, E] fp32
    m = logits.max(axis=1, keepdims=True)
    p = np.exp(logits - m, dtype=np.float32)
    p /= p.sum(axis=1, keepdims=True)
    # softmax is monotonic in logits, so top-2 by probs == top-2 by logits
    top_i = np.argsort(-p, axis=1, kind="stable")[:, :TOPK]  # [T, 2]
    top_p = np.take_along_axis(p, top_i, axis=1)
    gate_weights = top_p / (top_p.sum(axis=1, keepdims=True) + np.float32(1e-8))
    return top_i, gate_weights.astype(np.float32)


def kernel(x, gate_w, Wg, W1, W2):
    from concourse.bass_utils import run_bass_kernel_spmd

    x = np.asarray(x, dtype=np.float32)
    gate_w = np.asarray(gate_w, dtype=np.float32)
    Wg = np.asarray(Wg, dtype=np.float32)
    W1 = np.asarray(W1, dtype=np.float32)
    W2 = np.asarray(W2, dtype=np.float32)

    xf = x.reshape(-1, D)
    top_i, gate_weights = _route(xf, gate_w)

    idx = [None] * E
    wts = [None] * E
    for e in range(E):
        rows, slots = np.nonzero(top_i == e)
        idx[e] = rows
        wts[e] = gate_weights[rows, slots]
    counts = np.array([len(i) for i in idx])
    C = max(128, int(np.ceil(counts.max() / 128)) * 128)

    nc = _build_program(C)

    xf_bf = xf.astype(_BF16)
    in_maps = []
    for e in range(E):
        xT_e = np.zeros((D, C), dtype=_BF16)
        xT_e[:, : counts[e]] = xf_bf[idx[e]].T
        wgT_e = np.zeros((D, HP), dtype=_BF16)
        wgT_e[:, :H] = Wg[e].astype(_BF16).T
        w1T_e = np.zeros((D, HP), dtype=_BF16)
        w1T_e[:, :H] = W1[e].astype(_BF16).T
        w2T_e = np.zeros((HP, D), dtype=_BF16)
        w2T_e[:H, :] = W2[e].astype(_BF16).T
        in_maps.append({"xT": xT_e, "wgT": wgT_e, "w1T": w1T_e, "w2T": w2T_e})

    res = run_bass_kernel_spmd(nc, in_maps, list(range(N_CORES)))

    out = np.zeros((T, D), dtype=np.float32)
    for e in range(E):
        yT_e = res.results[e]["yT"]  # [D, C] fp32
        out[idx[e]] += wts[e][:, None] * yT_e.T[: counts[e]]
    return out.reshape(B, S, D)
